# revision 1
# baseline (speedup 1.0000x reference)
"""GCNConv layer on 8 Trainium2 NeuronCores (Bass/Tile).

Strategy (graph/data parallel, dst-sharded):
  - 8 cores, each owns N/8 destination nodes (blocks of 128).
  - Full nfeat (bf16) replicated to every core's HBM; per-core edges are
    gathered with dma_gather (SWDGE), so no collectives are needed.
  - Host (numpy) does index-side prep only: bucket edges by
    (core, dst-block, src-half), sort, pad each segment to a uniform
    tile count, build one-hot helper index arrays.
  - On device, per 4-block chunk: DVE builds every dst one-hot in one
    wide is_equal op; TensorE matmul-scatters gathered messages into a
    slice-chained PSUM region (feature-major).  Edge-feature sums and
    degrees are pure index data, so the host ships a per-dst vocab-count
    matrix (cmat) and a broadcast 1/(deg+1) (rdegb): efeat = emb.T @
    cmat via one matmul per block, then one chunk-wide epilogue computes
    y = (nfeat + neigh) * rdegb and out = W.T @ y + b.  Gather calls are
    trimmed per (block, src-half) group to the max-over-cores edge count
    so padding costs almost no gather payload.
  - int16 gather indices only reach 32767, so nfeat is split into two
    N/2-row tables (lo/hi src halves gathered separately).

Outputs are produced feature-major [128, NPAD] per core and
transposed/concatenated on the host.
"""
import sys

if "/opt/trn_rl_repo" not in sys.path:
    sys.path.insert(0, "/opt/trn_rl_repo")

import numpy as np
import ml_dtypes

import concourse.bass as bass
import concourse.mybir as mybir
import concourse.tile as tile
from concourse import bacc
from concourse.bass_utils import run_bass_kernel_spmd

bf16 = mybir.dt.bfloat16
f32 = mybir.dt.float32
i16 = mybir.dt.int16
npbf = ml_dtypes.bfloat16

D = 128
M = 8                 # cores
CHUNK = 4             # dst blocks per gather call pair

_cache = {}
AMP = 1  # bench-only: repeat the compute body AMP times (amplified timing)
ABLATE = set()  # perf-model ablation flags
DEBUG_TAPS = False  # extra DRAM outputs per stage
GATHER_CALL_TILES = 8  # <=8 tiles (1024 idxs) keeps single_packet mode


def _build(T, N, npc, nblk, lens):
    """Build + compile the SPMD kernel for T tiles per segment.

    lens[blk][seg] = gather length (multiple of 16) for that group, the
    max edge count over the 8 cores -- identical across cores, so the
    SPMD program stays uniform while skipping most pad-slot payload.
    """
    key = (T, N, npc, nblk, lens, AMP)
    if key in _cache:
        return _cache[key]

    TPB = 2 * T                      # tiles per block
    NT = nblk * TPB                  # tiles per core
    NE_SLOTS = NT * 128              # edge slots per core
    SEG = T * 128                    # slots per segment
    npad = nblk * 128
    split = N // 2
    nchunks = (nblk + CHUNK - 1) // CHUNK

    nc = bacc.Bacc("TRN2", target_bir_lowering=False, debug=False)

    d_tabA = nc.dram_tensor("tabA", [split, D], bf16, kind="ExternalInput").ap()
    d_tabB = nc.dram_tensor("tabB", [N - split, D], bf16, kind="ExternalInput").ap()
    d_idx = nc.dram_tensor("idx", [128, NE_SLOTS // 16], i16, kind="ExternalInput").ap()
    d_dstrel = nc.dram_tensor("dstrel", [128, NT], bf16, kind="ExternalInput").ap()
    d_iota = nc.dram_tensor("iota", [128, 128], bf16, kind="ExternalInput").ap()
    d_emb = nc.dram_tensor("emb", [32, D], bf16, kind="ExternalInput").ap()
    d_cmat = nc.dram_tensor("cmat", [32, npad], bf16, kind="ExternalInput").ap()
    d_rdegb = nc.dram_tensor("rdegb", [128, npad], bf16, kind="ExternalInput").ap()
    d_nfT = nc.dram_tensor("nfT", [128, npad], bf16, kind="ExternalInput").ap()
    d_W = nc.dram_tensor("W", [D, D], bf16, kind="ExternalInput").ap()
    d_b = nc.dram_tensor("b", [D, 1], f32, kind="ExternalInput").ap()
    d_out = nc.dram_tensor("out", [D, npad], f32, kind="ExternalOutput").ap()
    if DEBUG_TAPS:
        d_cnt = nc.dram_tensor("dbg_cnt", [32, npad], f32, kind="ExternalOutput").ap()
        d_y = nc.dram_tensor("dbg_y", [D, npad], f32, kind="ExternalOutput").ap()
        d_fm = nc.dram_tensor("dbg_fm", [D, npad], f32, kind="ExternalOutput").ap()
        d_rdeg = nc.dram_tensor("dbg_rdeg", [D, npad], f32, kind="ExternalOutput").ap()

    with tile.TileContext(nc) as tc:
        with (
            tc.tile_pool(name="const", bufs=1) as cpool,
            tc.tile_pool(name="gather", bufs=3) as gpool,
            tc.tile_pool(name="oh", bufs=2) as ohpool,
            tc.tile_pool(name="ep", bufs=3) as eppool,
            tc.tile_pool(name="psum", bufs=2, space="PSUM") as ppool,
            tc.tile_pool(name="psum_out", bufs=2, space="PSUM") as popool,
        ):
            t_idx = cpool.tile([128, NE_SLOTS // 16], i16)
            nc.sync.dma_start(t_idx[:], d_idx[:])
            t_dstrel = cpool.tile([128, NT], bf16)
            nc.sync.dma_start(t_dstrel[:], d_dstrel[:])
            t_iota = cpool.tile([128, 128], bf16)
            nc.sync.dma_start(t_iota[:], d_iota[:])
            t_emb = cpool.tile([32, D], bf16)
            nc.sync.dma_start(t_emb[:], d_emb[:])
            t_cmat = cpool.tile([32, npad], bf16)
            nc.sync.dma_start(t_cmat[:], d_cmat[:])
            t_rdegb = cpool.tile([128, npad], bf16)
            nc.sync.dma_start(t_rdegb[:], d_rdegb[:])
            t_W = cpool.tile([D, D], bf16)
            nc.sync.dma_start(t_W[:], d_W[:])
            t_b = cpool.tile([D, 1], f32)
            nc.sync.dma_start(t_b[:], d_b[:])

            # dma_gather with single_packet=True is limited to 1024 indices;
            # one call per (block, src-half) group, trimmed to the group's
            # max-over-cores edge count. Tiles beyond the trimmed length keep
            # stale SBUF content, which the all-zero one-hot columns mask.
            def gather_group(g, tile0, tab, slot0, n16):
                if "gather" in ABLATE or n16 == 0:
                    return
                ntc = (n16 + 127) // 128
                nc.gpsimd.dma_gather(
                    g[:, tile0:tile0 + ntc, :], tab,
                    t_idx[:, slot0 // 16:slot0 // 16 + n16 // 16],
                    n16, n16, D,
                    single_packet=(n16 <= 1024),
                )

            for c in [cc for _rep in range(AMP) for cc in range(nchunks)]:
                nb = min(CHUNK, nblk - c * CHUNK)
                chunk_slot0 = c * CHUNK * TPB * 128  # first slot of chunk
                g = gpool.tile([128, CHUNK * TPB, 128], bf16, tag="g")
                if c < 3:
                    # first rotation of the pool: zero-init so untouched pad
                    # tiles can't hold NaN bit patterns (0*NaN != 0)
                    nc.vector.memset(g[:].rearrange("p a b -> p (a b)"), 0.0)
                for j in range(nb):
                    gather_group(g, j * T, d_tabA[:],
                                 chunk_slot0 + j * T * 128,
                                 lens[c * CHUNK + j][0])
                    gather_group(g, nb * T + j * T, d_tabB[:],
                                 chunk_slot0 + (nb * T + j * T) * 128,
                                 lens[c * CHUNK + j][1])
                blk0 = c * CHUNK
                wid = nb * 128
                psum_fm = ppool.tile([128, CHUNK * 128], f32, tag="fm")
                gtile0 = c * CHUNK * TPB
                oh = ohpool.tile([128, CHUNK * TPB, 128], bf16, tag="ohd")
                if "ohd" not in ABLATE:
                    # one wide build: all dst one-hots for this chunk
                    nc.vector.tensor_tensor(
                        out=oh[:, 0:nb * TPB, :],
                        in0=t_dstrel[:, gtile0:gtile0 + nb * TPB]
                            .rearrange("p (t o) -> p t o", o=1)
                            .to_broadcast([128, nb * TPB, 128]),
                        in1=t_iota[:].rearrange("p (o e) -> p o e", o=1)
                            .to_broadcast([128, nb * TPB, 128]),
                        op=mybir.AluOpType.is_equal,
                    )
                for j in range(nb):
                    blk = blk0 + j
                    fmj = psum_fm[:, j * 128:(j + 1) * 128]
                    for t in range(TPB):
                        seg, ts_ = (0, t) if t < T else (1, t - T)
                        slot = seg * nb * T + j * T + ts_
                        first = t == 0
                        if "mm" not in ABLATE:
                            nc.tensor.matmul(
                                out=fmj, lhsT=g[:, slot, :], rhs=oh[:, slot, :],
                                start=first, stop=False,
                            )
                    nc.tensor.matmul(
                        out=fmj, lhsT=t_emb[:],
                        rhs=t_cmat[:, blk * 128:(blk + 1) * 128],
                        start=False, stop=True,
                    )
                # chunk-wide epilogue over nb blocks at once
                nfT_ch = eppool.tile([128, CHUNK * 128], bf16, tag="nfT")
                nc.sync.dma_start(nfT_ch[:, :wid],
                                  d_nfT[:, blk0 * 128:blk0 * 128 + wid])
                ysum = eppool.tile([128, CHUNK * 128], f32, tag="ysum")
                nc.vector.tensor_tensor(
                    out=ysum[:, :wid], in0=psum_fm[:, :wid], in1=nfT_ch[:, :wid],
                    op=mybir.AluOpType.add,
                )
                y = eppool.tile([128, CHUNK * 128], bf16, tag="y")
                nc.vector.tensor_tensor(
                    out=y[:, :wid], in0=ysum[:, :wid],
                    in1=t_rdegb[:, blk0 * 128:blk0 * 128 + wid],
                    op=mybir.AluOpType.mult,
                )
                psum_out = popool.tile([128, CHUNK * 128], f32, tag="po")
                nc.tensor.matmul(
                    out=psum_out[:, :wid], lhsT=t_W[:], rhs=y[:, :wid],
                    start=True, stop=True,
                )
                out_sb = eppool.tile([128, CHUNK * 128], f32, tag="osb")
                nc.vector.tensor_scalar_add(out_sb[:, :wid], psum_out[:, :wid],
                                            t_b[:, 0:1])
                nc.sync.dma_start(
                    d_out[:, blk0 * 128:blk0 * 128 + wid], out_sb[:, :wid]
                )

    nc.compile()
    _cache[key] = nc
    return nc


def prepare(nfeat, src, dst, efeat_idx, edge_emb, W, b):
    """Host-side prep: returns (nc, in_maps, assembler)."""
    nfeat = np.asarray(nfeat, np.float32)
    src = np.asarray(src, np.int64)
    dst = np.asarray(dst, np.int64)
    efeat_idx = np.asarray(efeat_idx, np.int64)
    edge_emb = np.asarray(edge_emb, np.float32)
    W = np.asarray(W, np.float32)
    b = np.asarray(b, np.float32)

    N, _ = nfeat.shape
    E = src.shape[0]
    NF, V, _ = edge_emb.shape
    npc = N // M
    nblk = (npc + 127) // 128
    npad = nblk * 128
    split = N // 2

    core = dst // npc
    dst_local = dst % npc
    blk = dst_local // 128
    rel = (dst_local % 128).astype(np.float32)
    seg = (src >= split).astype(np.int64)

    # group id = ((core*nblk + blk)*2 + seg); rank of edge within group
    gid = (core * nblk + blk) * 2 + seg
    order = np.argsort(gid, kind="stable")
    gsorted = gid[order]
    counts = np.bincount(gid, minlength=M * nblk * 2)
    starts = np.concatenate([[0], np.cumsum(counts)[:-1]])
    rank = np.empty(E, np.int64)
    rank[order] = np.arange(E) - starts[gsorted]

    T = max(1, int((counts.max() + 127) // 128))
    TPB = 2 * T
    NT = nblk * TPB
    NE_SLOTS = NT * 128

    # slot of each edge within its core's slot space
    c_of_blk = blk // CHUNK
    j_of_blk = blk % CHUNK
    nb_of_blk = np.minimum(CHUNK, nblk - c_of_blk * CHUNK)
    chunk_slot0 = c_of_blk * CHUNK * TPB * 128
    slot = chunk_slot0 + (seg * nb_of_blk * T + j_of_blk * T) * 128 + rank

    # per-core packed arrays
    idx_all = np.zeros((M, NE_SLOTS), np.int16)
    dstrel_all = np.full((M, NE_SLOTS), -1.0, np.float32)
    idx_all[core, slot] = (src - seg * split).astype(np.int16)
    dstrel_all[core, slot] = rel

    # host-computed per-dst count matrix (vocab slots 8..31) and 1/(deg+1)
    dst_local_pad = core * npad + blk * 128 + (dst_local % 128)
    cmat_all = np.zeros((32, M * npad), np.float32)
    for c_ in range(NF):
        np.add.at(cmat_all, (8 + c_ * V + efeat_idx[:, c_], dst_local_pad), 1.0)
    deg_all = np.zeros(M * npad, np.float32)
    np.add.at(deg_all, dst_local_pad, 1.0)
    rdeg_all = 1.0 / (deg_all + 1.0)

    nfeat_bf = nfeat.astype(npbf)
    tabA = np.ascontiguousarray(nfeat_bf[:split])
    tabB = np.ascontiguousarray(nfeat_bf[split:])
    iota_b = np.tile(np.arange(128, dtype=np.float32)[None, :], (128, 1)).astype(npbf)
    emb32 = np.zeros((32, D), np.float32)
    emb32[8:8 + NF * V] = edge_emb.reshape(NF * V, D)
    emb32 = emb32.astype(npbf)
    W_bf = W.astype(npbf)
    b_col = b.reshape(D, 1).astype(np.float32)

    in_maps = []
    for k in range(M):
        idx_w = np.tile(
            np.ascontiguousarray(idx_all[k].reshape(NE_SLOTS // 16, 16).T), (8, 1)
        )
        dstrelT = np.ascontiguousarray(
            dstrel_all[k].reshape(NT, 128).T
        ).astype(npbf)
        nfT = np.zeros((128, npad), npbf)
        nfT[:, :npc] = nfeat_bf[k * npc:(k + 1) * npc].T
        cmat_k = np.ascontiguousarray(
            cmat_all[:, k * npad:(k + 1) * npad]).astype(npbf)
        rdegb_k = np.ascontiguousarray(np.tile(
            rdeg_all[k * npad:(k + 1) * npad][None, :], (128, 1))).astype(npbf)
        in_maps.append({
            "tabA": tabA, "tabB": tabB, "idx": idx_w, "dstrel": dstrelT,
            "iota": iota_b, "emb": emb32, "cmat": cmat_k, "rdegb": rdegb_k,
            "nfT": np.ascontiguousarray(nfT), "W": W_bf, "b": b_col,
        })

    # per-(block, src-half) gather length: max edge count over cores,
    # rounded up to the 16-index descriptor-lane granularity
    gmax = counts.reshape(M, nblk, 2).max(axis=0)
    lens = tuple(
        (int(-(-int(gmax[b_, 0]) // 16) * 16), int(-(-int(gmax[b_, 1]) // 16) * 16))
        for b_ in range(nblk)
    )

    nc = _build(T, N, npc, nblk, lens)

    def assemble(results):
        out = np.empty((N, D), np.float32)
        for k in range(M):
            out[k * npc:(k + 1) * npc] = results[k]["out"][:, :npc].T
        return out

    return nc, in_maps, assemble


def kernel(nfeat, src, dst, efeat_idx, edge_emb, W, b):
    nc, in_maps, assemble = prepare(nfeat, src, dst, efeat_idx, edge_emb, W, b)
    res = run_bass_kernel_spmd(nc, in_maps, core_ids=list(range(M)))
    return assemble(res.results)



# revision 9
# speedup vs baseline: 674.8736x; 674.8736x over previous
"""GCNConv layer on 8 Trainium2 NeuronCores (Bass/Tile).

Strategy (graph/data parallel, dst-sharded):
  - 8 cores, each owns N/8 destination nodes (blocks of 128 dsts).
  - Full nfeat (bf16) replicated per core in HBM as two 32768-row
    tables (int16 gather indices); per-edge src rows are fetched with
    gpsimd.dma_gather spread across all 4 SWDGE queues (the single
    default queue serializes at ~26 GB/s; 4 queues reach ~105 GB/s).
  - Packed tile layout: each (dst-block, src-half) group gets exactly
    ceil(maxcount/128) slots-tiles laid back-to-back; the src-half
    split is balanced per (core, block) through the [N-32768, 32768)
    overlap window so seg0 groups land on exact 128-multiples.
  - Edges sorted by src within each group for HBM row locality.
  - DVE builds all dst one-hots of a 7-block chunk in one wide
    is_equal; TensorE matmul-scatters gathered messages into PSUM
    (feat-major); edge-feature sums ship as a per-dst vocab-count
    matrix (cmat) folded in via one matmul per block.
  - Epilogue: psum_out[d,fo] = (fm+nfT).T@W + (deg+1)·b via three
    accumulating matmuls per block (d-major), degree normalization is
    a per-partition Activation-engine scale, and the output is written
    d-major so host assembly is a plain row copy.
"""
import sys

if "/opt/trn_rl_repo" not in sys.path:
    sys.path.insert(0, "/opt/trn_rl_repo")

import numpy as np
import ml_dtypes

import concourse.bass as bass
import concourse.mybir as mybir
import concourse.tile as tile
from concourse import bacc
from concourse.bass_utils import run_bass_kernel_spmd

bf16 = mybir.dt.bfloat16
f32 = mybir.dt.float32
i16 = mybir.dt.int16
npbf = ml_dtypes.bfloat16

D = 128
M = 8                 # cores
CH = 7                # dst blocks per chunk (49 = 7*7)

_cache = {}
AMP = 1
ABLATE = set()
NQUEUES = 4
SINGLE_PACKET = False
GBUFS = 3
GSPAN = 1  # dst blocks covered per gather call (1..CH); CH = one call per (chunk, seg)
FORCE_SP = None  # None -> auto (sn16 <= 1024); True/False force single_packet
SCRATCH = 16384  # dynamic_dma_scratch_size (SWDGE descriptor ring space)
BALANCE = True  # balance src-half split via the int16 overlap window
FULLG = False  # True: full-tile gathers on first GBUFS chunks; False: memset
SORT_SRC = True  # sort edges by src within each (block, seg) group
GORDER = "s"  # gather issue order: "j" block-major, "s" seg-major
SALT = 0  # cache-bust for recompile tests


def _layout(tiles):
    """tiles[b][s] -> per-chunk tile offsets.

    Returns (NT, chunk_info) where chunk_info[c] = dict with
      tile0: first global tile of chunk
      ntiles: tiles in chunk
      goff[(j, s)]: local tile offset of group (block c*CH+j, seg s)
      gcall[s]: (local_tile0, ntiles_s, n16_s) for the merged gather
    """
    nblk = len(tiles)
    nchunks = (nblk + CH - 1) // CH
    chunk_info = []
    gt = 0
    for c in range(nchunks):
        nb = min(CH, nblk - c * CH)
        info = {"tile0": gt, "goff": {}, "gcall": {}}
        local = 0
        for s in (0, 1):
            s_t0 = local
            s_n16 = 0
            for j in range(nb):
                b = c * CH + j
                info["goff"][(j, s)] = local
                local += tiles[b][s][0]
                s_n16 = (local - s_t0 - tiles[b][s][0]) * 128 + tiles[b][s][1]
            info["gcall"][s] = (s_t0, local - s_t0, s_n16)
        info["ntiles"] = local
        gt += local
        chunk_info.append(info)
    return gt, chunk_info


def _build(N, npc, nblk, tiles):
    key = (N, npc, nblk, tiles, AMP, NQUEUES, SINGLE_PACKET, GBUFS, GSPAN,
           FORCE_SP, SCRATCH, FULLG, BALANCE, GORDER, SALT,
           tuple(sorted(ABLATE)))
    if key in _cache:
        return _cache[key]

    NT, chunk_info = _layout(tiles)
    NE_SLOTS = NT * 128
    npad = nblk * 128
    tabrows = min(N, 32768) if BALANCE else -(-N // 2)
    nchunks = (nblk + CH - 1) // CH
    CTMAX = max(ci["ntiles"] for ci in chunk_info)

    nc = bacc.Bacc("TRN2", target_bir_lowering=False, debug=False,
                   num_swdge_queues=NQUEUES,
                   dynamic_dma_scratch_size=SCRATCH)

    d_tabA = nc.dram_tensor("tabA", [tabrows, D], bf16, kind="ExternalInput").ap()
    d_tabB = nc.dram_tensor("tabB", [tabrows, D], bf16, kind="ExternalInput").ap()
    d_idx = nc.dram_tensor("idx", [128, NE_SLOTS // 16], i16, kind="ExternalInput").ap()
    d_dstrel = nc.dram_tensor("dstrel", [128, NT], bf16, kind="ExternalInput").ap()
    d_iota = nc.dram_tensor("iota", [128, 128], bf16, kind="ExternalInput").ap()
    d_emb = nc.dram_tensor("emb", [32, D], bf16, kind="ExternalInput").ap()
    d_cmat = nc.dram_tensor("cmat", [32, npad], bf16, kind="ExternalInput").ap()
    d_nfT = nc.dram_tensor("nfT", [128, npad], bf16, kind="ExternalInput").ap()
    d_W = nc.dram_tensor("W", [D, D], bf16, kind="ExternalInput").ap()
    d_brow = nc.dram_tensor("brow", [1, D], bf16, kind="ExternalInput").ap()
    d_dginv = nc.dram_tensor("dginv", [1, npad], bf16, kind="ExternalInput").ap()
    d_rdeg = nc.dram_tensor("rdeg", [128, nblk], f32, kind="ExternalInput").ap()
    d_out = nc.dram_tensor("out", [npad, D], f32, kind="ExternalOutput").ap()

    with tile.TileContext(nc) as tc:
        with (
            tc.tile_pool(name="const", bufs=1) as cpool,
            tc.tile_pool(name="gather", bufs=GBUFS) as gpool,
            tc.tile_pool(name="oh", bufs=2) as ohpool,
            tc.tile_pool(name="ep", bufs=2) as eppool,
            tc.tile_pool(name="psum", bufs=2, space="PSUM") as ppool,
            tc.tile_pool(name="psum_out", bufs=2, space="PSUM") as popool,
        ):
            t_idx = cpool.tile([128, NE_SLOTS // 16], i16)
            nc.sync.dma_start(t_idx[:], d_idx[:])
            t_dstrel = cpool.tile([128, NT], bf16)
            nc.sync.dma_start(t_dstrel[:], d_dstrel[:])
            t_iota = cpool.tile([128, 128], bf16)
            nc.sync.dma_start(t_iota[:], d_iota[:])
            t_emb = cpool.tile([32, D], bf16)
            nc.sync.dma_start(t_emb[:], d_emb[:])
            t_cmat = cpool.tile([32, npad], bf16)
            nc.sync.dma_start(t_cmat[:], d_cmat[:])
            t_W = cpool.tile([D, D], bf16)
            nc.sync.dma_start(t_W[:], d_W[:])
            t_brow = cpool.tile([1, D], bf16)
            nc.sync.dma_start(t_brow[:], d_brow[:])
            t_dginv = cpool.tile([1, npad], bf16)
            nc.sync.dma_start(t_dginv[:], d_dginv[:])
            t_rdeg = cpool.tile([128, nblk], f32)
            nc.sync.dma_start(t_rdeg[:], d_rdeg[:])

            gq = [0]
            for c in [cc for _rep in range(AMP) for cc in range(nchunks)]:
                ci = chunk_info[c]
                nb = min(CH, nblk - c * CH)
                cn = ci["ntiles"]
                blk0 = c * CH
                wid = nb * 128

                g = gpool.tile([128, CTMAX, 128], bf16, tag="g")
                if not FULLG and c < GBUFS:
                    nc.vector.memset(g[:].rearrange("p a b -> p (a b)"), 0.0)
                if "gather" not in ABLATE:
                    if GORDER == "j":
                        calls = [(j0, s) for j0 in range(0, nb, GSPAN)
                                 for s in (0, 1)]
                    else:
                        calls = [(j0, s) for s in (0, 1)
                                 for j0 in range(0, nb, GSPAN)]
                    for j0, s in calls:
                        tab = d_tabA if s == 0 else d_tabB
                        if True:
                            jlast = min(j0 + GSPAN, nb) - 1
                            lt0 = ci["goff"][(j0, s)]
                            ltend = ci["goff"][(jlast, s)]
                            snt = ltend - lt0 + tiles[blk0 + jlast][s][0]
                            if FULLG and c < GBUFS:
                                # first pool rotations: gather full tiles so
                                # no slot holds uninitialized SBUF (pad idx=0
                                # rows are masked by the zero one-hot cols)
                                sn16 = snt * 128
                            else:
                                sn16 = ((ltend - lt0) * 128
                                        + tiles[blk0 + jlast][s][1])
                            if sn16 == 0:
                                continue
                            slot0 = (ci["tile0"] + lt0) * 128
                            sp = (sn16 <= 1024) if FORCE_SP is None else FORCE_SP
                            nc.gpsimd.dma_gather(
                                g[:, lt0:lt0 + snt, :], tab[:],
                                t_idx[:, slot0 // 16:slot0 // 16 + sn16 // 16],
                                sn16, sn16, D,
                                single_packet=sp,
                                queue_num=gq[0] % NQUEUES,
                            )
                            gq[0] += 1

                oh = ohpool.tile([128, CTMAX, 128], bf16, tag="oh")
                if "ohd" not in ABLATE:
                    nc.vector.tensor_tensor(
                        out=oh[:, 0:cn, :],
                        in0=t_dstrel[:, ci["tile0"]:ci["tile0"] + cn]
                            .rearrange("p (t o) -> p t o", o=1)
                            .to_broadcast([128, cn, 128]),
                        in1=t_iota[:].rearrange("p (o e) -> p o e", o=1)
                            .to_broadcast([128, cn, 128]),
                        op=mybir.AluOpType.is_equal,
                    )

                psum_fm = ppool.tile([128, CH * 128], f32, tag="fm")
                for j in range(nb):
                    blk = blk0 + j
                    fmj = psum_fm[:, j * 128:(j + 1) * 128]
                    first = True
                    if "mm" not in ABLATE:
                        for s in (0, 1):
                            t0 = ci["goff"][(j, s)]
                            for t in range(tiles[blk][s][0]):
                                nc.tensor.matmul(
                                    out=fmj, lhsT=g[:, t0 + t, :],
                                    rhs=oh[:, t0 + t, :],
                                    start=first, stop=False,
                                )
                                first = False
                    nc.tensor.matmul(
                        out=fmj, lhsT=t_emb[:],
                        rhs=t_cmat[:, blk * 128:(blk + 1) * 128],
                        start=first, stop=True,
                    )

                fm_sb = eppool.tile([128, CH * 128], bf16, tag="fmsb")
                nc.scalar.copy(fm_sb[:, :wid], psum_fm[:, :wid])
                nfT_ch = eppool.tile([128, CH * 128], bf16, tag="nfT")
                nc.sync.dma_start(nfT_ch[:, :wid],
                                  d_nfT[:, blk0 * 128:blk0 * 128 + wid])

                psum_out = popool.tile([128, CH * 128], f32, tag="po")
                for j in range(nb):
                    blk = blk0 + j
                    poj = psum_out[:, j * 128:(j + 1) * 128]
                    nc.tensor.matmul(
                        out=poj, lhsT=fm_sb[:, j * 128:(j + 1) * 128],
                        rhs=t_W[:], start=True, stop=False,
                    )
                    nc.tensor.matmul(
                        out=poj, lhsT=nfT_ch[:, j * 128:(j + 1) * 128],
                        rhs=t_W[:], start=False, stop=False,
                    )
                    nc.tensor.matmul(
                        out=poj, lhsT=t_dginv[:, blk * 128:(blk + 1) * 128],
                        rhs=t_brow[:], start=False, stop=True,
                    )

                out_sb = eppool.tile([128, CH * 128], f32, tag="osb")
                for j in range(nb):
                    blk = blk0 + j
                    nc.scalar.mul(
                        out_sb[:, j * 128:(j + 1) * 128],
                        psum_out[:, j * 128:(j + 1) * 128],
                        t_rdeg[:, blk:blk + 1],
                    )
                nc.sync.dma_start(
                    d_out[blk0 * 128:blk0 * 128 + wid, :]
                        .rearrange("(j p) f -> p j f", p=128),
                    out_sb[:, :wid].rearrange("p (j f) -> p j f", f=128),
                )

    nc.compile()
    _cache[key] = nc
    return nc


def prepare(nfeat, src, dst, efeat_idx, edge_emb, W, b):
    nfeat = np.asarray(nfeat, np.float32)
    src = np.asarray(src, np.int64)
    dst = np.asarray(dst, np.int64)
    efeat_idx = np.asarray(efeat_idx, np.int64)
    edge_emb = np.asarray(edge_emb, np.float32)
    W = np.asarray(W, np.float32)
    b = np.asarray(b, np.float32)

    N, _ = nfeat.shape
    E = src.shape[0]
    NF, V, _ = edge_emb.shape
    npc = N // M
    nblk = (npc + 127) // 128
    npad = nblk * 128
    split = N // 2

    core = dst // npc
    dst_local = dst % npc
    blk = dst_local // 128
    rel = (dst_local % 128).astype(np.float32)

    # Balanced src-half split: tabA = nfeat[:32768], tabB = nfeat[N-32768:].
    # Edges with src in the overlap [N-32768, 32768) can use either table;
    # assign them so seg0's per-(core,block) count tops up to a 128-multiple
    # X_b >= max-core forced0 count, minimizing total tiles.
    tabrows = min(N, 32768) if BALANCE else -(-N // 2)
    base1 = N - tabrows
    key_cb = core * nblk + blk
    if base1 > 0 and BALANCE:
        forced0 = src < base1
        forced1 = src >= tabrows
        free = ~forced0 & ~forced1
        a_cb = np.bincount(key_cb[forced0], minlength=M * nblk)
        t_cb = np.bincount(key_cb, minlength=M * nblk)
        amax_b = a_cb.reshape(M, nblk).max(axis=0)
        X_b = 128 * (-(-amax_b // 128))
        quota_cb = np.maximum(
            0, np.minimum(X_b[None, :], t_cb.reshape(M, nblk)) - a_cb.reshape(M, nblk)
        ).reshape(-1)
        fidx = np.flatnonzero(free)
        kf = key_cb[fidx]
        of = np.argsort(kf, kind="stable")
        countsf = np.bincount(kf, minlength=M * nblk)
        startsf = np.concatenate([[0], np.cumsum(countsf)[:-1]])
        rankf = np.empty(len(fidx), np.int64)
        rankf[of] = np.arange(len(fidx)) - startsf[kf[of]]
        seg = np.zeros(E, np.int64)
        seg[forced1] = 1
        seg[fidx] = (rankf >= quota_cb[kf]).astype(np.int64)
    else:
        seg = (src >= tabrows).astype(np.int64)

    gid = (core * nblk + blk) * 2 + seg
    order = np.lexsort((src, gid)) if SORT_SRC else np.argsort(gid, kind="stable")
    gsorted = gid[order]
    counts = np.bincount(gid, minlength=M * nblk * 2)
    starts = np.concatenate([[0], np.cumsum(counts)[:-1]])
    rank = np.empty(E, np.int64)
    rank[order] = np.arange(E) - starts[gsorted]

    gmax = counts.reshape(M, nblk, 2).max(axis=0)
    n16 = -(-gmax // 16) * 16
    ntile = -(-n16 // 128)
    tiles = tuple(
        ((int(ntile[b_, 0]), int(n16[b_, 0])),
         (int(ntile[b_, 1]), int(n16[b_, 1])))
        for b_ in range(nblk)
    )

    NT, chunk_info = _layout(tiles)
    NE_SLOTS = NT * 128

    # slot of each edge: group tile offset (global) * 128 + rank
    goff_global = np.zeros((nblk, 2), np.int64)
    for b_ in range(nblk):
        c_, j_ = b_ // CH, b_ % CH
        ci = chunk_info[c_]
        goff_global[b_, 0] = ci["tile0"] + ci["goff"][(j_, 0)]
        goff_global[b_, 1] = ci["tile0"] + ci["goff"][(j_, 1)]
    slot = goff_global[blk, seg] * 128 + rank

    idx_all = np.zeros((M, NE_SLOTS), np.int16)
    dstrel_all = np.full((M, NE_SLOTS), -1.0, np.float32)
    idx_all[core, slot] = (src - seg * base1).astype(np.int16)
    dstrel_all[core, slot] = rel

    dst_local_pad = core * npad + blk * 128 + (dst_local % 128)
    cmat_all = np.zeros((32, M * npad), np.float32)
    for c_ in range(NF):
        np.add.at(cmat_all, (8 + c_ * V + efeat_idx[:, c_], dst_local_pad), 1.0)
    deg_all = np.zeros(M * npad, np.float32)
    np.add.at(deg_all, dst_local_pad, 1.0)
    dginv_all = deg_all + 1.0
    rdeg_all = 1.0 / dginv_all

    nfeat_bf = nfeat.astype(npbf)
    tabA = np.ascontiguousarray(nfeat_bf[:tabrows])
    tabB = np.ascontiguousarray(nfeat_bf[base1:])
    iota_b = np.tile(np.arange(128, dtype=np.float32)[None, :], (128, 1)).astype(npbf)
    emb32 = np.zeros((32, D), np.float32)
    emb32[8:8 + NF * V] = edge_emb.reshape(NF * V, D)
    emb32 = emb32.astype(npbf)
    W_bf = W.astype(npbf)
    b_row = b.reshape(1, D).astype(npbf)

    in_maps = []
    for k in range(M):
        idx_w = np.tile(
            np.ascontiguousarray(idx_all[k].reshape(NE_SLOTS // 16, 16).T), (8, 1)
        )
        dstrelT = np.ascontiguousarray(
            dstrel_all[k].reshape(NT, 128).T
        ).astype(npbf)
        nfT = np.zeros((128, npad), npbf)
        nfT[:, :npc] = nfeat_bf[k * npc:(k + 1) * npc].T
        cmat_k = np.ascontiguousarray(
            cmat_all[:, k * npad:(k + 1) * npad]).astype(npbf)
        dginv_k = np.ascontiguousarray(
            dginv_all[k * npad:(k + 1) * npad][None, :]).astype(npbf)
        rdeg_k = np.ascontiguousarray(
            rdeg_all[k * npad:(k + 1) * npad].reshape(nblk, 128).T
        ).astype(np.float32)
        in_maps.append({
            "tabA": tabA, "tabB": tabB, "idx": idx_w, "dstrel": dstrelT,
            "iota": iota_b, "emb": emb32, "cmat": cmat_k,
            "nfT": np.ascontiguousarray(nfT), "W": W_bf, "brow": b_row,
            "dginv": dginv_k, "rdeg": rdeg_k,
        })

    nc = _build(N, npc, nblk, tiles)

    def assemble(results):
        out = np.empty((N, D), np.float32)
        for k in range(M):
            out[k * npc:(k + 1) * npc] = results[k]["out"][:npc]
        return out

    return nc, in_maps, assemble


def kernel(nfeat, src, dst, efeat_idx, edge_emb, W, b):
    nc, in_maps, assemble = prepare(nfeat, src, dst, efeat_idx, edge_emb, W, b)
    res = run_bass_kernel_spmd(nc, in_maps, core_ids=list(range(M)))
    return assemble(res.results)


# revision 10
# speedup vs baseline: 852.6333x; 1.2634x over previous
"""GCNConv layer on 8 Trainium2 NeuronCores (Bass/Tile).

Strategy (graph/data parallel, dst-sharded):
  - 8 cores, each owns N/8 destination nodes (blocks of 128 dsts).
  - Full nfeat (bf16) replicated per core in HBM as two 32768-row
    tables (int16 gather indices); per-edge src rows are fetched with
    gpsimd.dma_gather spread across all 4 SWDGE queues (a single
    queue drains at ~26 GB/s; 4 queues reach ~105 GB/s).
  - Packed tile layout: each (dst-block, src-half) group gets exactly
    ceil(maxcount/128) slot-tiles laid back-to-back; the src-half
    split is balanced per (core, block) through the [N-32768, 32768)
    overlap window so seg0 groups land on exact 128-multiples.
  - Edges sorted by src within each group for HBM row locality;
    gather-pool memsets are hoisted ahead of the chunk loop so they
    overlap the constant-table loads.
  - DVE builds all dst one-hots of a 7-block chunk in one wide
    is_equal; TensorE matmul-scatters gathered messages into PSUM
    (feat-major); edge-feature sums ship as a per-dst vocab-count
    matrix (cmat) folded in via one matmul per block.
  - Epilogue: psum_out[d,fo] = (fm+nfT).T@W + (deg+1)*b via three
    accumulating matmuls per block (d-major), degree normalization is
    a per-partition Activation-engine scale, and the output is written
    d-major so host assembly is a plain row copy.
"""
import sys

if "/opt/trn_rl_repo" not in sys.path:
    sys.path.insert(0, "/opt/trn_rl_repo")

import numpy as np
import ml_dtypes

import concourse.bass as bass
import concourse.mybir as mybir
import concourse.tile as tile
from concourse import bacc
from concourse.bass_utils import run_bass_kernel_spmd

bf16 = mybir.dt.bfloat16
f32 = mybir.dt.float32
i16 = mybir.dt.int16
npbf = ml_dtypes.bfloat16

D = 128
M = 8                 # cores
CH = 7                # dst blocks per chunk (49 = 7*7)

_cache = {}
AMP = 1
ABLATE = set()
NQUEUES = 4
SINGLE_PACKET = False
GBUFS = 3
GSPAN = 1  # dst blocks covered per gather call (1..CH); CH = one call per (chunk, seg)
FORCE_SP = None  # None -> auto (sn16 <= 1024); True/False force single_packet
SCRATCH = 16384  # dynamic_dma_scratch_size (SWDGE descriptor ring space)
BALANCE = True  # balance src-half split via the int16 overlap window
FULLG = False  # True: full-tile gathers on first GBUFS chunks; False: memset
SORT_SRC = True  # sort edges by src within each (block, seg) group
GORDER = "s"  # gather issue order: "j" block-major, "s" seg-major
OHBUFS = 2
EPBUFS = 2
MSHOIST = True  # memset gather bufs before the chunk loop (overlaps const loads)
SALT = 0  # cache-bust for recompile tests


def _layout(tiles):
    """tiles[b][s] -> per-chunk tile offsets.

    Returns (NT, chunk_info) where chunk_info[c] = dict with
      tile0: first global tile of chunk
      ntiles: tiles in chunk
      goff[(j, s)]: local tile offset of group (block c*CH+j, seg s)
      gcall[s]: (local_tile0, ntiles_s, n16_s) for the merged gather
    """
    nblk = len(tiles)
    nchunks = (nblk + CH - 1) // CH
    chunk_info = []
    gt = 0
    for c in range(nchunks):
        nb = min(CH, nblk - c * CH)
        info = {"tile0": gt, "goff": {}, "gcall": {}}
        local = 0
        for s in (0, 1):
            s_t0 = local
            s_n16 = 0
            for j in range(nb):
                b = c * CH + j
                info["goff"][(j, s)] = local
                local += tiles[b][s][0]
                s_n16 = (local - s_t0 - tiles[b][s][0]) * 128 + tiles[b][s][1]
            info["gcall"][s] = (s_t0, local - s_t0, s_n16)
        info["ntiles"] = local
        gt += local
        chunk_info.append(info)
    return gt, chunk_info


def _build(N, npc, nblk, tiles):
    key = (N, npc, nblk, tiles, AMP, NQUEUES, SINGLE_PACKET, GBUFS, GSPAN,
           FORCE_SP, SCRATCH, FULLG, BALANCE, GORDER, SALT, OHBUFS, EPBUFS,
           MSHOIST, tuple(sorted(ABLATE)))
    if key in _cache:
        return _cache[key]

    NT, chunk_info = _layout(tiles)
    NE_SLOTS = NT * 128
    npad = nblk * 128
    tabrows = min(N, 32768) if BALANCE else -(-N // 2)
    nchunks = (nblk + CH - 1) // CH
    CTMAX = max(ci["ntiles"] for ci in chunk_info)

    nc = bacc.Bacc("TRN2", target_bir_lowering=False, debug=False,
                   num_swdge_queues=NQUEUES,
                   dynamic_dma_scratch_size=SCRATCH)

    d_tabA = nc.dram_tensor("tabA", [tabrows, D], bf16, kind="ExternalInput").ap()
    d_tabB = nc.dram_tensor("tabB", [tabrows, D], bf16, kind="ExternalInput").ap()
    d_idx = nc.dram_tensor("idx", [128, NE_SLOTS // 16], i16, kind="ExternalInput").ap()
    d_dstrel = nc.dram_tensor("dstrel", [128, NT], bf16, kind="ExternalInput").ap()
    d_iota = nc.dram_tensor("iota", [128, 128], bf16, kind="ExternalInput").ap()
    d_emb = nc.dram_tensor("emb", [32, D], bf16, kind="ExternalInput").ap()
    d_cmat = nc.dram_tensor("cmat", [32, npad], bf16, kind="ExternalInput").ap()
    d_nfT = nc.dram_tensor("nfT", [128, npad], bf16, kind="ExternalInput").ap()
    d_W = nc.dram_tensor("W", [D, D], bf16, kind="ExternalInput").ap()
    d_brow = nc.dram_tensor("brow", [1, D], bf16, kind="ExternalInput").ap()
    d_dginv = nc.dram_tensor("dginv", [1, npad], bf16, kind="ExternalInput").ap()
    d_rdeg = nc.dram_tensor("rdeg", [128, nblk], f32, kind="ExternalInput").ap()
    d_out = nc.dram_tensor("out", [npad, D], f32, kind="ExternalOutput").ap()

    with tile.TileContext(nc) as tc:
        with (
            tc.tile_pool(name="const", bufs=1) as cpool,
            tc.tile_pool(name="gather", bufs=GBUFS) as gpool,
            tc.tile_pool(name="oh", bufs=OHBUFS) as ohpool,
            tc.tile_pool(name="ep", bufs=EPBUFS) as eppool,
            tc.tile_pool(name="psum", bufs=2, space="PSUM") as ppool,
            tc.tile_pool(name="psum_out", bufs=2, space="PSUM") as popool,
        ):
            t_idx = cpool.tile([128, NE_SLOTS // 16], i16)
            nc.sync.dma_start(t_idx[:], d_idx[:])
            t_dstrel = cpool.tile([128, NT], bf16)
            nc.sync.dma_start(t_dstrel[:], d_dstrel[:])
            t_iota = cpool.tile([128, 128], bf16)
            nc.sync.dma_start(t_iota[:], d_iota[:])
            t_emb = cpool.tile([32, D], bf16)
            nc.sync.dma_start(t_emb[:], d_emb[:])
            t_cmat = cpool.tile([32, npad], bf16)
            nc.sync.dma_start(t_cmat[:], d_cmat[:])
            t_W = cpool.tile([D, D], bf16)
            nc.sync.dma_start(t_W[:], d_W[:])
            t_brow = cpool.tile([1, D], bf16)
            nc.sync.dma_start(t_brow[:], d_brow[:])
            t_dginv = cpool.tile([1, npad], bf16)
            nc.sync.dma_start(t_dginv[:], d_dginv[:])
            t_rdeg = cpool.tile([128, nblk], f32)
            nc.sync.dma_start(t_rdeg[:], d_rdeg[:])

            if not FULLG and MSHOIST:
                # init the gather pool up front so the memsets overlap the
                # constant-table DMA loads instead of stalling chunk 0
                for _i in range(GBUFS):
                    gi = gpool.tile([128, CTMAX, 128], bf16, tag="g")
                    nc.vector.memset(gi[:].rearrange("p a b -> p (a b)"), 0.0)

            gq = [0]
            for c in [cc for _rep in range(AMP) for cc in range(nchunks)]:
                ci = chunk_info[c]
                nb = min(CH, nblk - c * CH)
                cn = ci["ntiles"]
                blk0 = c * CH
                wid = nb * 128

                g = gpool.tile([128, CTMAX, 128], bf16, tag="g")
                if not FULLG and not MSHOIST and c < GBUFS:
                    nc.vector.memset(g[:].rearrange("p a b -> p (a b)"), 0.0)
                if "gather" not in ABLATE:
                    if GORDER == "j":
                        calls = [(j0, s) for j0 in range(0, nb, GSPAN)
                                 for s in (0, 1)]
                    else:
                        calls = [(j0, s) for s in (0, 1)
                                 for j0 in range(0, nb, GSPAN)]
                    for j0, s in calls:
                        tab = d_tabA if s == 0 else d_tabB
                        if True:
                            jlast = min(j0 + GSPAN, nb) - 1
                            lt0 = ci["goff"][(j0, s)]
                            ltend = ci["goff"][(jlast, s)]
                            snt = ltend - lt0 + tiles[blk0 + jlast][s][0]
                            if FULLG and c < GBUFS:
                                # first pool rotations: gather full tiles so
                                # no slot holds uninitialized SBUF (pad idx=0
                                # rows are masked by the zero one-hot cols)
                                sn16 = snt * 128
                            else:
                                sn16 = ((ltend - lt0) * 128
                                        + tiles[blk0 + jlast][s][1])
                            if sn16 == 0:
                                continue
                            slot0 = (ci["tile0"] + lt0) * 128
                            sp = (sn16 <= 1024) if FORCE_SP is None else FORCE_SP
                            nc.gpsimd.dma_gather(
                                g[:, lt0:lt0 + snt, :], tab[:],
                                t_idx[:, slot0 // 16:slot0 // 16 + sn16 // 16],
                                sn16, sn16, D,
                                single_packet=sp,
                                queue_num=gq[0] % NQUEUES,
                            )
                            gq[0] += 1

                oh = ohpool.tile([128, CTMAX, 128], bf16, tag="oh")
                if "ohd" not in ABLATE:
                    nc.vector.tensor_tensor(
                        out=oh[:, 0:cn, :],
                        in0=t_dstrel[:, ci["tile0"]:ci["tile0"] + cn]
                            .rearrange("p (t o) -> p t o", o=1)
                            .to_broadcast([128, cn, 128]),
                        in1=t_iota[:].rearrange("p (o e) -> p o e", o=1)
                            .to_broadcast([128, cn, 128]),
                        op=mybir.AluOpType.is_equal,
                    )

                psum_fm = ppool.tile([128, CH * 128], f32, tag="fm")
                for j in range(nb):
                    blk = blk0 + j
                    fmj = psum_fm[:, j * 128:(j + 1) * 128]
                    first = True
                    if "mm" not in ABLATE:
                        for s in (0, 1):
                            t0 = ci["goff"][(j, s)]
                            for t in range(tiles[blk][s][0]):
                                nc.tensor.matmul(
                                    out=fmj, lhsT=g[:, t0 + t, :],
                                    rhs=oh[:, t0 + t, :],
                                    start=first, stop=False,
                                )
                                first = False
                    nc.tensor.matmul(
                        out=fmj, lhsT=t_emb[:],
                        rhs=t_cmat[:, blk * 128:(blk + 1) * 128],
                        start=first, stop=True,
                    )

                fm_sb = eppool.tile([128, CH * 128], bf16, tag="fmsb")
                nc.scalar.copy(fm_sb[:, :wid], psum_fm[:, :wid])
                nfT_ch = eppool.tile([128, CH * 128], bf16, tag="nfT")
                nc.sync.dma_start(nfT_ch[:, :wid],
                                  d_nfT[:, blk0 * 128:blk0 * 128 + wid])

                psum_out = popool.tile([128, CH * 128], f32, tag="po")
                for j in range(nb):
                    blk = blk0 + j
                    poj = psum_out[:, j * 128:(j + 1) * 128]
                    nc.tensor.matmul(
                        out=poj, lhsT=fm_sb[:, j * 128:(j + 1) * 128],
                        rhs=t_W[:], start=True, stop=False,
                    )
                    nc.tensor.matmul(
                        out=poj, lhsT=nfT_ch[:, j * 128:(j + 1) * 128],
                        rhs=t_W[:], start=False, stop=False,
                    )
                    nc.tensor.matmul(
                        out=poj, lhsT=t_dginv[:, blk * 128:(blk + 1) * 128],
                        rhs=t_brow[:], start=False, stop=True,
                    )

                out_sb = eppool.tile([128, CH * 128], f32, tag="osb")
                for j in range(nb):
                    blk = blk0 + j
                    nc.scalar.mul(
                        out_sb[:, j * 128:(j + 1) * 128],
                        psum_out[:, j * 128:(j + 1) * 128],
                        t_rdeg[:, blk:blk + 1],
                    )
                nc.sync.dma_start(
                    d_out[blk0 * 128:blk0 * 128 + wid, :]
                        .rearrange("(j p) f -> p j f", p=128),
                    out_sb[:, :wid].rearrange("p (j f) -> p j f", f=128),
                )

    nc.compile()
    _cache[key] = nc
    return nc


def prepare(nfeat, src, dst, efeat_idx, edge_emb, W, b):
    nfeat = np.asarray(nfeat, np.float32)
    src = np.asarray(src, np.int64)
    dst = np.asarray(dst, np.int64)
    efeat_idx = np.asarray(efeat_idx, np.int64)
    edge_emb = np.asarray(edge_emb, np.float32)
    W = np.asarray(W, np.float32)
    b = np.asarray(b, np.float32)

    N, _ = nfeat.shape
    E = src.shape[0]
    NF, V, _ = edge_emb.shape
    npc = N // M
    nblk = (npc + 127) // 128
    npad = nblk * 128
    split = N // 2

    core = dst // npc
    dst_local = dst % npc
    blk = dst_local // 128
    rel = (dst_local % 128).astype(np.float32)

    # Balanced src-half split: tabA = nfeat[:32768], tabB = nfeat[N-32768:].
    # Edges with src in the overlap [N-32768, 32768) can use either table;
    # assign them so seg0's per-(core,block) count tops up to a 128-multiple
    # X_b >= max-core forced0 count, minimizing total tiles.
    tabrows = min(N, 32768) if BALANCE else -(-N // 2)
    base1 = N - tabrows
    key_cb = core * nblk + blk
    if base1 > 0 and BALANCE:
        forced0 = src < base1
        forced1 = src >= tabrows
        free = ~forced0 & ~forced1
        a_cb = np.bincount(key_cb[forced0], minlength=M * nblk)
        t_cb = np.bincount(key_cb, minlength=M * nblk)
        amax_b = a_cb.reshape(M, nblk).max(axis=0)
        X_b = 128 * (-(-amax_b // 128))
        quota_cb = np.maximum(
            0, np.minimum(X_b[None, :], t_cb.reshape(M, nblk)) - a_cb.reshape(M, nblk)
        ).reshape(-1)
        fidx = np.flatnonzero(free)
        kf = key_cb[fidx]
        of = np.argsort(kf, kind="stable")
        countsf = np.bincount(kf, minlength=M * nblk)
        startsf = np.concatenate([[0], np.cumsum(countsf)[:-1]])
        rankf = np.empty(len(fidx), np.int64)
        rankf[of] = np.arange(len(fidx)) - startsf[kf[of]]
        seg = np.zeros(E, np.int64)
        seg[forced1] = 1
        seg[fidx] = (rankf >= quota_cb[kf]).astype(np.int64)
    else:
        seg = (src >= tabrows).astype(np.int64)

    gid = (core * nblk + blk) * 2 + seg
    order = np.lexsort((src, gid)) if SORT_SRC else np.argsort(gid, kind="stable")
    gsorted = gid[order]
    counts = np.bincount(gid, minlength=M * nblk * 2)
    starts = np.concatenate([[0], np.cumsum(counts)[:-1]])
    rank = np.empty(E, np.int64)
    rank[order] = np.arange(E) - starts[gsorted]

    gmax = counts.reshape(M, nblk, 2).max(axis=0)
    n16 = -(-gmax // 16) * 16
    ntile = -(-n16 // 128)
    tiles = tuple(
        ((int(ntile[b_, 0]), int(n16[b_, 0])),
         (int(ntile[b_, 1]), int(n16[b_, 1])))
        for b_ in range(nblk)
    )

    NT, chunk_info = _layout(tiles)
    NE_SLOTS = NT * 128

    # slot of each edge: group tile offset (global) * 128 + rank
    goff_global = np.zeros((nblk, 2), np.int64)
    for b_ in range(nblk):
        c_, j_ = b_ // CH, b_ % CH
        ci = chunk_info[c_]
        goff_global[b_, 0] = ci["tile0"] + ci["goff"][(j_, 0)]
        goff_global[b_, 1] = ci["tile0"] + ci["goff"][(j_, 1)]
    slot = goff_global[blk, seg] * 128 + rank

    idx_all = np.zeros((M, NE_SLOTS), np.int16)
    dstrel_all = np.full((M, NE_SLOTS), -1.0, np.float32)
    idx_all[core, slot] = (src - seg * base1).astype(np.int16)
    dstrel_all[core, slot] = rel

    dst_local_pad = core * npad + blk * 128 + (dst_local % 128)
    cmat_all = np.zeros((32, M * npad), np.float32)
    for c_ in range(NF):
        np.add.at(cmat_all, (8 + c_ * V + efeat_idx[:, c_], dst_local_pad), 1.0)
    deg_all = np.zeros(M * npad, np.float32)
    np.add.at(deg_all, dst_local_pad, 1.0)
    dginv_all = deg_all + 1.0
    rdeg_all = 1.0 / dginv_all

    nfeat_bf = nfeat.astype(npbf)
    tabA = np.ascontiguousarray(nfeat_bf[:tabrows])
    tabB = np.ascontiguousarray(nfeat_bf[base1:])
    iota_b = np.tile(np.arange(128, dtype=np.float32)[None, :], (128, 1)).astype(npbf)
    emb32 = np.zeros((32, D), np.float32)
    emb32[8:8 + NF * V] = edge_emb.reshape(NF * V, D)
    emb32 = emb32.astype(npbf)
    W_bf = W.astype(npbf)
    b_row = b.reshape(1, D).astype(npbf)

    in_maps = []
    for k in range(M):
        idx_w = np.tile(
            np.ascontiguousarray(idx_all[k].reshape(NE_SLOTS // 16, 16).T), (8, 1)
        )
        dstrelT = np.ascontiguousarray(
            dstrel_all[k].reshape(NT, 128).T
        ).astype(npbf)
        nfT = np.zeros((128, npad), npbf)
        nfT[:, :npc] = nfeat_bf[k * npc:(k + 1) * npc].T
        cmat_k = np.ascontiguousarray(
            cmat_all[:, k * npad:(k + 1) * npad]).astype(npbf)
        dginv_k = np.ascontiguousarray(
            dginv_all[k * npad:(k + 1) * npad][None, :]).astype(npbf)
        rdeg_k = np.ascontiguousarray(
            rdeg_all[k * npad:(k + 1) * npad].reshape(nblk, 128).T
        ).astype(np.float32)
        in_maps.append({
            "tabA": tabA, "tabB": tabB, "idx": idx_w, "dstrel": dstrelT,
            "iota": iota_b, "emb": emb32, "cmat": cmat_k,
            "nfT": np.ascontiguousarray(nfT), "W": W_bf, "brow": b_row,
            "dginv": dginv_k, "rdeg": rdeg_k,
        })

    nc = _build(N, npc, nblk, tiles)

    def assemble(results):
        out = np.empty((N, D), np.float32)
        for k in range(M):
            out[k * npc:(k + 1) * npc] = results[k]["out"][:npc]
        return out

    return nc, in_maps, assemble


def kernel(nfeat, src, dst, efeat_idx, edge_emb, W, b):
    nc, in_maps, assemble = prepare(nfeat, src, dst, efeat_idx, edge_emb, W, b)
    res = run_bass_kernel_spmd(nc, in_maps, core_ids=list(range(M)))
    return assemble(res.results)
